# revision 9
# baseline (speedup 1.0000x reference)
"""TRN2 Bass kernel for nn_Bionic_68092411510857 (3-modality GAT towers).

Design:
- host: dedup n_ids (unique < 32767 -> int16 dma_gather indices), transpose
  pre_W, exact jax replica of the interp() PRNG mask, per-core input slicing.
- device (8 cores as 4 pairs; pairs 0/1/2 own modalities, pair 3 duplicates
  modality 2 with zeroed coefficients):
  phase A: gather unique WpreT rows (lo/hi split tables with zero dummy row),
    x_l = xg @ WgT + b via data-stationary matmuls (row output), s =
    exp(leakyrelu(alpha)), store prescaled rows [y=s*x_l | s | pad] fp16.
  L0: dma_gather 768B rows by edge, degree-16 segment sum via 16 selector
    matmuls accumulating into one PSUM tile, g0 = numer/denom + bias,
    xm0 = coeff*g0, build X2 rows (y2|s2) the same way.
  pair AllGather X2; L1 same pattern over all 4096 dsts (coeff halves),
  xmod read-modify-write; ReduceScatter xmod; emb shard; AllGather emb;
  dot row-shard per core.
"""
import numpy as np

P = 128
NCORES = 8
IN_SIZE = 50000
M = 3
N0 = 40960
N1 = 16384
B = 4096
HEADS = 4
DIM = 64
HD = 256
DEG = 16
E0 = N1 * DEG
E1 = B * DEG

NUP = 28672            # padded unique-node table rows (224 tiles)
RWE = 384              # fp16 row elems -> 768B
LO_ROWS = 32768        # 1 zero row + 32767 WpreT rows
HI_ROWS = 17280        # 1 zero row + 17233 WpreT rows (padded)
NI = 2048              # idxs per dma_gather call
D0SH = 8192            # L0 dsts per core
L0CH = D0SH * DEG // NI      # 64
ACH = NUP // NI              # 14
L1CH = B * DEG // NI         # 32
X2R = 16384
DOTSH = B // NCORES          # 512

_CACHE = {}


def _interp_host(masks, scales_param):
    import jax, jax.numpy as jnp
    cpu = jax.devices("cpu")[0]
    with jax.default_device(cpu):
        scales = jax.nn.softmax(jnp.asarray(scales_param), axis=-1)
        mask = jnp.asarray(masks)
        key = jax.random.key(42)
        rm = jax.random.randint(key, mask.shape, 0, 2).astype(jnp.float32)
        rm = rm + (1.0 / (1.0 + rm.sum(-1)) ** 20)[:, None]
        rm = rm + (1.0 / (mask.sum(-1) ** 20))[:, None]
        rm = jnp.floor(rm)
        rm = rm / (rm + 1e-10)
        m = mask * rm
        m = jax.nn.softmax(m + (1.0 - m) * (-1e10), axis=-1)
        return np.asarray(scales), np.asarray(m)


def _wrap16(idx, n):
    """idx int array len n (n%16==0) -> [128, n//16] int16 wrapped+replicated."""
    a = np.asarray(idx, dtype=np.int16).reshape(n // 16, 16).T  # [16, n/16]
    return np.tile(a, (8, 1)).copy()


def _build_nc():
    import concourse.bass as bass
    import concourse.bacc as bacc
    import concourse.mybir as mybir
    import concourse.tile as tile

    f16 = mybir.dt.float16
    f32 = mybir.dt.float32
    i16 = mybir.dt.int16
    AL = mybir.AluOpType

    nc = bacc.Bacc(None, num_devices=NCORES)

    # ---- inputs ----
    wlo = nc.declare_dram_parameter("wlo", [LO_ROWS, HD], f16, isOutput=False)
    whi = nc.declare_dram_parameter("whi", [HI_ROWS, HD], f16, isOutput=False)
    galo = nc.declare_dram_parameter("galo", [P, NUP // 16], i16, isOutput=False)
    gahi = nc.declare_dram_parameter("gahi", [P, NUP // 16], i16, isOutput=False)
    e0i = nc.declare_dram_parameter("e0i", [P, D0SH * DEG // 16], i16, isOutput=False)
    e1i = nc.declare_dram_parameter("e1i", [P, E1 // 16], i16, isOutput=False)
    wgt = nc.declare_dram_parameter("wgt", [2, P, HD], f16, isOutput=False)
    wa = nc.declare_dram_parameter("wa", [2, P, HEADS], f16, isOutput=False)
    bwg = nc.declare_dram_parameter("bwg", [1, HD], f16, isOutput=False)
    ba = nc.declare_dram_parameter("ba", [1, HEADS], f16, isOutput=False)
    gbt = nc.declare_dram_parameter("gbt", [P, HD], f32, isOutput=False)
    ct0 = nc.declare_dram_parameter("ct0", [P, 32], f32, isOutput=False)
    ct1 = nc.declare_dram_parameter("ct1", [P, 32], f32, isOutput=False)
    embwt = nc.declare_dram_parameter("embwt", [2, P, P], f32, isOutput=False)
    embb = nc.declare_dram_parameter("embb", [P, 1], f32, isOutput=False)
    idn16 = nc.declare_dram_parameter("idn16", [P, P], f16, isOutput=False)
    idn32 = nc.declare_dram_parameter("idn32", [P, P], f32, isOutput=False)
    sel = nc.declare_dram_parameter("sel", [P, 16, P], f16, isOutput=False)

    # ---- outputs ----
    dot_sh = nc.declare_dram_parameter("dot_sh", [DOTSH, B], f32, isOutput=True)
    emb_sh = nc.declare_dram_parameter("emb_sh", [DOTSH, P], f32, isOutput=True)

    # ---- internal DRAM ----
    X = nc.dram_tensor("X", [NUP, RWE], f16)
    X2in = nc.dram_tensor("X2in", [D0SH, RWE], f16)
    X2 = nc.dram_tensor("X2", [X2R, RWE], f16)
    xmod = nc.dram_tensor("xmod", [B, HD], f32)
    xmod_rs = nc.dram_tensor("xmod_rs", [DOTSH, HD], f32)
    embag_in = nc.dram_tensor("embag_in", [DOTSH, P], f32)
    emb_full = nc.dram_tensor("emb_full", [B, P], f32)

    with tile.TileContext(nc) as tc:
        cp = tc.tile_pool(name="consts", bufs=1)
        cpool = cp.__enter__()
        # ---- constants in SBUF ----
        c_wgt = cpool.tile([P, 2, HD], f16)
        nc.sync.dma_start(c_wgt[:], wgt[:].rearrange("b p c -> p b c"))
        c_wa = cpool.tile([P, 2, HEADS], f16)
        nc.sync.dma_start(c_wa[:], wa[:].rearrange("b p c -> p b c"))
        c_bwg = cpool.tile([1, HD], f16)
        nc.sync.dma_start(c_bwg[:], bwg[:])
        c_ba = cpool.tile([1, HEADS], f16)
        nc.sync.dma_start(c_ba[:], ba[:])
        c_gbt = cpool.tile([P, HD], f32)
        nc.sync.dma_start(c_gbt[:], gbt[:])
        c_ct0 = cpool.tile([P, 32], f32)
        nc.sync.dma_start(c_ct0[:], ct0[:])
        c_ct1 = cpool.tile([P, 32], f32)
        nc.sync.dma_start(c_ct1[:], ct1[:])
        c_ewt = cpool.tile([P, 2, P], f32)
        nc.sync.dma_start(c_ewt[:], embwt[:].rearrange("b p c -> p b c"))
        c_ebb = cpool.tile([P, 1], f32)
        nc.sync.dma_start(c_ebb[:], embb[:])
        c_id16 = cpool.tile([P, P], f16)
        nc.sync.dma_start(c_id16[:], idn16[:])
        c_id32 = cpool.tile([P, P], f32)
        nc.sync.dma_start(c_id32[:], idn32[:])
        c_sel = cpool.tile([P, 16, P], f16)
        nc.sync.dma_start(c_sel[:], sel[:])
        c_galo = cpool.tile([P, NUP // 16], i16)
        nc.sync.dma_start(c_galo[:], galo[:])
        c_gahi = cpool.tile([P, NUP // 16], i16)
        nc.sync.dma_start(c_gahi[:], gahi[:])
        c_e0 = cpool.tile([P, D0SH * DEG // 16], i16)
        nc.sync.dma_start(c_e0[:], e0i[:])
        c_e1 = cpool.tile([P, E1 // 16], i16)
        nc.sync.dma_start(c_e1[:], e1i[:])
        c_ones = cpool.tile([1, P], f16)
        nc.vector.memset(c_ones[:], 1.0)
        zt = cpool.tile([P, 2048], f32)
        nc.vector.memset(zt[:], 0.0)
        # absorbers: pull const-DMA waits onto DVE with 2D ops
        ab = cpool.tile([P, 8], f32)
        nc.vector.tensor_copy(out=ab[:, 0:2], in_=c_ct0[:, 0:2])
        nc.vector.tensor_copy(out=ab[:, 2:4], in_=c_gbt[:, 0:2])
        ab2 = cpool.tile([P, 8], f16)
        nc.vector.tensor_copy(out=ab2[:, 0:2], in_=c_wgt[:, 0, 0:2])
        nc.vector.tensor_copy(out=ab2[:, 2:4], in_=c_sel[:, 0, 0:2])
        nc.vector.tensor_copy(out=ab2[:, 4:6], in_=c_id16[:, 0:2])

        # zero xmod
        for z in range(4):
            nc.sync.dma_start(
                xmod[z * 1024:(z + 1) * 1024, :].rearrange("(g p) c -> p g c", p=P),
                zt[:].rearrange("p (g c) -> p g c", c=HD),
            )

        # ================= PHASE A =================
        with tc.tile_pool(name="pa", bufs=2) as pa, \
             tc.tile_pool(name="pap", bufs=2, space="PSUM") as pap, \
             tc.tile_pool(name="paq", bufs=1, space="PSUM") as paq, \
             tc.tile_pool(name="pas", bufs=3) as pas:
            for ch in range(ACH):
                glo = pa.tile([P, NI // P, HD], f16, tag="glo")
                nc.gpsimd.dma_gather(
                    out_ap=glo[:], in_ap=wlo[:, :],
                    idxs_ap=c_galo[:, ch * NI // 16:(ch + 1) * NI // 16],
                    num_idxs=NI, num_idxs_reg=NI, elem_size=HD,
                    transpose=False, single_packet=False,
                )
                ghi = pa.tile([P, NI // P, HD], f16, tag="ghi")
                nc.gpsimd.dma_gather(
                    out_ap=ghi[:], in_ap=whi[:, :],
                    idxs_ap=c_gahi[:, ch * NI // 16:(ch + 1) * NI // 16],
                    num_idxs=NI, num_idxs_reg=NI, elem_size=HD,
                    transpose=False, single_packet=False,
                )
                xg = pa.tile([P, NI // P, HD], f16, tag="xg")
                nc.vector.tensor_tensor(out=xg[:], in0=glo[:], in1=ghi[:], op=AL.add)
                # per 128-node tile j: node (p, j) is unique-row ch*NI + j*128 + p
                for j in range(NI // P):
                    tp0 = paq.tile([P, P], f16, space="PSUM", tag="tp0")
                    nc.tensor.transpose(out=tp0[:], in_=xg[:, j, 0:P], identity=c_id16[:])
                    tp1 = paq.tile([P, P], f16, space="PSUM", tag="tp1")
                    nc.tensor.transpose(out=tp1[:], in_=xg[:, j, P:HD], identity=c_id16[:])
                    xgt = pas.tile([P, 2, P], f16, tag="xgt")
                    nc.vector.tensor_copy(out=xgt[:, 0, :], in_=tp0[:])
                    nc.scalar.copy(out=xgt[:, 1, :], in_=tp1[:])
                    pxl = pap.tile([P, HD], f32, space="PSUM", tag="pxl")
                    nc.tensor.matmul(pxl[:], lhsT=xgt[:, 0, :], rhs=c_wgt[:, 0, :], start=True, stop=False)
                    nc.tensor.matmul(pxl[:], lhsT=xgt[:, 1, :], rhs=c_wgt[:, 1, :], start=False, stop=False)
                    nc.tensor.matmul(pxl[:], lhsT=c_ones[:], rhs=c_bwg[:], start=False, stop=True)
                    pal = paq.tile([P, HEADS], f32, space="PSUM", tag="pal")
                    nc.tensor.matmul(pal[:], lhsT=xgt[:, 0, :], rhs=c_wa[:, 0, :], start=True, stop=False)
                    nc.tensor.matmul(pal[:], lhsT=xgt[:, 1, :], rhs=c_wa[:, 1, :], start=False, stop=False)
                    nc.tensor.matmul(pal[:], lhsT=c_ones[:], rhs=c_ba[:], start=False, stop=True)
                    # s = exp(leaky_relu(alpha))
                    sal = pas.tile([P, HEADS], f32, tag="sal")
                    nc.scalar.activation(sal[:], pal[:],
                                         mybir.ActivationFunctionType.Lrelu, alpha=0.2)
                    sex = pas.tile([P, HEADS], f32, tag="sex")
                    nc.scalar.activation(sex[:], sal[:],
                                         mybir.ActivationFunctionType.Exp)
                    row = pas.tile([P, RWE], f16, tag="row")
                    nc.vector.tensor_tensor(
                        out=row[:, 0:HD].rearrange("p (h c) -> p h c", h=HEADS),
                        in0=pxl[:].rearrange("p (h c) -> p h c", h=HEADS),
                        in1=sex[:].to_broadcast([P, HEADS, DIM]),
                        op=AL.mult,
                    )
                    nc.scalar.copy(out=row[:, HD:HD + HEADS], in_=sex[:])
                    nc.vector.memset(row[:, HD + HEADS:RWE], 0.0)
                    nc.sync.dma_start(X[ch * NI + j * P:(ch * NI + (j + 1) * P), :], row[:])

        # ================= L0 =================
        with tc.tile_pool(name="l0", bufs=3) as l0, \
             tc.tile_pool(name="l0p", bufs=2, space="PSUM") as l0p, \
             tc.tile_pool(name="l0q", bufs=1, space="PSUM") as l0q, \
             tc.tile_pool(name="l0s", bufs=3) as l0s:
            for ch in range(L0CH):
                G = l0.tile([P, NI // P, RWE], f16, tag="G")
                nc.gpsimd.dma_gather(
                    out_ap=G[:], in_ap=X[:, :],
                    idxs_ap=c_e0[:, ch * NI // 16:(ch + 1) * NI // 16],
                    num_idxs=NI, num_idxs_reg=NI, elem_size=RWE,
                    transpose=False, single_packet=False,
                )
                pg = l0p.tile([P, HD + HEADS], f32, space="PSUM", tag="pg")
                for c in range(NI // P):
                    nc.tensor.matmul(
                        pg[:],
                        lhsT=c_sel[:, c, :], rhs=G[:, c, 0:HD + HEADS],
                        start=(c == 0), stop=(c == NI // P - 1),
                    )
                dinv = l0s.tile([P, HEADS], f32, tag="dinv")
                nc.vector.reciprocal(out=dinv[:], in_=pg[:, HD:HD + HEADS])
                g0 = l0s.tile([P, HD], f32, tag="g0")
                nc.vector.tensor_tensor(
                    out=g0[:].rearrange("p (h c) -> p h c", h=HEADS),
                    in0=pg[:, 0:HD].rearrange("p (h c) -> p h c", h=HEADS),
                    in1=dinv[:].to_broadcast([P, HEADS, DIM]),
                    op=AL.mult,
                )
                nc.vector.tensor_tensor(out=g0[:], in0=g0[:], in1=c_gbt[:], op=AL.add)
                if ch < 32:
                    xm = l0s.tile([P, HD], f32, tag="xm")
                    nc.vector.scalar_tensor_tensor(
                        out=xm[:], in0=g0[:], scalar=c_ct0[:, ch:ch + 1], in1=g0[:],
                        op0=AL.mult, op1=AL.bypass,
                    )
                    nc.sync.dma_start(xmod[ch * P:(ch + 1) * P, :], xm[:])
                # X2 row build
                t0 = l0q.tile([P, P], f32, space="PSUM", tag="t0")
                nc.tensor.transpose(out=t0[:], in_=g0[:, 0:P], identity=c_id32[:])
                t1 = l0q.tile([P, P], f32, space="PSUM", tag="t1")
                nc.tensor.transpose(out=t1[:], in_=g0[:, P:HD], identity=c_id32[:])
                g0t = l0s.tile([P, 2, P], f16, tag="g0t")
                nc.vector.tensor_copy(out=g0t[:, 0, :], in_=t0[:])
                nc.scalar.copy(out=g0t[:, 1, :], in_=t1[:])
                pxl2 = l0p.tile([P, HD], f32, space="PSUM", tag="pxl2")
                nc.tensor.matmul(pxl2[:], lhsT=g0t[:, 0, :], rhs=c_wgt[:, 0, :], start=True, stop=False)
                nc.tensor.matmul(pxl2[:], lhsT=g0t[:, 1, :], rhs=c_wgt[:, 1, :], start=False, stop=True)
                pal2 = l0q.tile([P, HEADS], f32, space="PSUM", tag="pal2")
                nc.tensor.matmul(pal2[:], lhsT=g0t[:, 0, :], rhs=c_wa[:, 0, :], start=True, stop=False)
                nc.tensor.matmul(pal2[:], lhsT=g0t[:, 1, :], rhs=c_wa[:, 1, :], start=False, stop=True)
                sal2 = l0s.tile([P, HEADS], f32, tag="sal2")
                nc.scalar.activation(sal2[:], pal2[:],
                                     mybir.ActivationFunctionType.Lrelu, alpha=0.2)
                sex2 = l0s.tile([P, HEADS], f32, tag="sex2")
                nc.scalar.activation(sex2[:], sal2[:], mybir.ActivationFunctionType.Exp)
                row2 = l0s.tile([P, RWE], f16, tag="row2")
                nc.vector.tensor_tensor(
                    out=row2[:, 0:HD].rearrange("p (h c) -> p h c", h=HEADS),
                    in0=pxl2[:].rearrange("p (h c) -> p h c", h=HEADS),
                    in1=sex2[:].to_broadcast([P, HEADS, DIM]),
                    op=AL.mult,
                )
                nc.scalar.copy(out=row2[:, HD:HD + HEADS], in_=sex2[:])
                nc.vector.memset(row2[:, HD + HEADS:RWE], 0.0)
                nc.sync.dma_start(X2in[ch * P:(ch + 1) * P, :], row2[:])

        # ================= X2 AllGather (pairs) =================
        nc.gpsimd.collective_compute(
            "AllGather", mybir.AluOpType.bypass,
            replica_groups=[[0, 1], [2, 3], [4, 5], [6, 7]],
            ins=[X2in[:]], outs=[X2[:]],
        )

        # ================= L1 =================
        with tc.tile_pool(name="l1", bufs=3) as l1, \
             tc.tile_pool(name="l1p", bufs=2, space="PSUM") as l1p, \
             tc.tile_pool(name="l1s", bufs=3) as l1s:
            for ch in range(L1CH):
                G = l1.tile([P, NI // P, RWE], f16, tag="G1")
                nc.gpsimd.dma_gather(
                    out_ap=G[:], in_ap=X2[:, :],
                    idxs_ap=c_e1[:, ch * NI // 16:(ch + 1) * NI // 16],
                    num_idxs=NI, num_idxs_reg=NI, elem_size=RWE,
                    transpose=False, single_packet=False,
                )
                pg = l1p.tile([P, HD + HEADS], f32, space="PSUM", tag="pg1")
                for c in range(NI // P):
                    nc.tensor.matmul(
                        pg[:],
                        lhsT=c_sel[:, c, :], rhs=G[:, c, 0:HD + HEADS],
                        start=(c == 0), stop=(c == NI // P - 1),
                    )
                dinv = l1s.tile([P, HEADS], f32, tag="dinv1")
                nc.vector.reciprocal(out=dinv[:], in_=pg[:, HD:HD + HEADS])
                g1 = l1s.tile([P, HD], f32, tag="g1")
                nc.vector.tensor_tensor(
                    out=g1[:].rearrange("p (h c) -> p h c", h=HEADS),
                    in0=pg[:, 0:HD].rearrange("p (h c) -> p h c", h=HEADS),
                    in1=dinv[:].to_broadcast([P, HEADS, DIM]),
                    op=AL.mult,
                )
                nc.vector.tensor_tensor(out=g1[:], in0=g1[:], in1=c_gbt[:], op=AL.add)
                # xm1 = coeff1 * g1; read-modify-write xmod
                xm_old = l1s.tile([P, HD], f32, tag="xm_old")
                nc.sync.dma_start(xm_old[:], xmod[ch * P:(ch + 1) * P, :])
                xm1 = l1s.tile([P, HD], f32, tag="xm1")
                nc.vector.scalar_tensor_tensor(
                    out=xm1[:], in0=g1[:], scalar=c_ct1[:, ch:ch + 1], in1=xm_old[:],
                    op0=AL.mult, op1=AL.add,
                )
                nc.sync.dma_start(xmod[ch * P:(ch + 1) * P, :], xm1[:])

        # ================= ReduceScatter xmod =================
        nc.gpsimd.collective_compute(
            "ReduceScatter", mybir.AluOpType.add,
            replica_groups=[list(range(NCORES))],
            ins=[xmod[:]], outs=[xmod_rs[:]],
        )

        # ================= emb shard =================
        with tc.tile_pool(name="em", bufs=2) as em, \
             tc.tile_pool(name="emp", bufs=1, space="PSUM") as emp:
            xsh = em.tile([P, 4, HD], f32, tag="xsh")
            nc.sync.dma_start(xsh[:], xmod_rs[:].rearrange("(a p) c -> p a c", p=P))
            xmt = em.tile([P, 2, DOTSH], f32, tag="xmt")
            for j in range(4):
                q0 = emp.tile([P, P], f32, space="PSUM", tag="q0")
                nc.tensor.transpose(out=q0[:], in_=xsh[:, j, 0:P], identity=c_id32[:])
                q1 = emp.tile([P, P], f32, space="PSUM", tag="q1")
                nc.tensor.transpose(out=q1[:], in_=xsh[:, j, P:HD], identity=c_id32[:])
                nc.vector.tensor_copy(out=xmt[:, 0, j * P:(j + 1) * P], in_=q0[:])
                nc.scalar.copy(out=xmt[:, 1, j * P:(j + 1) * P], in_=q1[:])
            pet = emp.tile([P, DOTSH], f32, space="PSUM", tag="pet")
            nc.tensor.matmul(pet[:], lhsT=c_ewt[:, 0, :], rhs=xmt[:, 0, :], start=True, stop=False)
            nc.tensor.matmul(pet[:], lhsT=c_ewt[:, 1, :], rhs=xmt[:, 1, :], start=False, stop=True)
            # embT (+bias) in f32
            embt0 = em.tile([P, DOTSH], f32, tag="embt0")
            nc.vector.tensor_copy(out=embt0[:], in_=pet[:])
            embt = em.tile([P, DOTSH], f32, tag="embt")
            nc.vector.scalar_tensor_tensor(
                out=embt[:], in0=embt0[:], scalar=c_ebb[:], in1=embt0[:],
                op0=AL.add, op1=AL.bypass,
            )
            # emb rows (transpose back) -> emb_sh output + AG input
            erow = em.tile([P, 4, P], f32, tag="erow")
            for j in range(4):
                q2 = emp.tile([P, P], f32, space="PSUM", tag="q2")
                nc.tensor.transpose(out=q2[:], in_=embt[:, j * P:(j + 1) * P], identity=c_id32[:])
                nc.vector.tensor_copy(out=erow[:, j, :], in_=q2[:])
            nc.sync.dma_start(emb_sh[:].rearrange("(a p) c -> p a c", p=P), erow[:])
            nc.sync.dma_start(embag_in[:].rearrange("(a p) c -> p a c", p=P), erow[:])

            nc.gpsimd.collective_compute(
                "AllGather", mybir.AluOpType.bypass,
                replica_groups=[list(range(NCORES))],
                ins=[embag_in[:]], outs=[emb_full[:]],
            )

            # ================= dot =================
            ef = em.tile([P, 32, P], f32, tag="ef")
            nc.sync.dma_start(ef[:], emb_full[:].rearrange("(a p) c -> p a c", p=P))
            eft = em.tile([P, B], f32, tag="eft")
            for j in range(32):
                q3 = emp.tile([P, P], f32, space="PSUM", tag="q3")
                nc.tensor.transpose(out=q3[:], in_=ef[:, j, :], identity=c_id32[:])
                if j % 2 == 0:
                    nc.vector.tensor_copy(out=eft[:, j * P:(j + 1) * P], in_=q3[:])
                else:
                    nc.scalar.copy(out=eft[:, j * P:(j + 1) * P], in_=q3[:])
            for mb in range(4):
                for nchk in range(8):
                    pd = emp.tile([P, 512], f32, space="PSUM", tag="pd")
                    nc.tensor.matmul(
                        pd[:],
                        lhsT=embt[:, mb * P:(mb + 1) * P],
                        rhs=eft[:, nchk * 512:(nchk + 1) * 512],
                        start=True, stop=True,
                    )
                    ds = em.tile([P, 512], f32, tag="ds")
                    nc.vector.tensor_copy(out=ds[:], in_=pd[:])
                    nc.sync.dma_start(
                        dot_sh[mb * P:(mb + 1) * P, nchk * 512:(nchk + 1) * 512], ds[:])
        cp.__exit__(None, None, None)
    nc.compile()
    return nc


def _host_prep(inputs):
    masks = np.asarray(inputs["masks"], np.float32)
    n_ids = np.asarray(inputs["n_ids"], np.int32)
    e0_src = np.asarray(inputs["e0_src"], np.int32)
    e1_src = np.asarray(inputs["e1_src"], np.int32)
    pre_W = np.asarray(inputs["pre_W"], np.float32)
    pre_b = np.asarray(inputs["pre_b"], np.float32)
    gat_W = np.asarray(inputs["gat_W"], np.float32)
    gat_att = np.asarray(inputs["gat_att"], np.float32)
    gat_b = np.asarray(inputs["gat_b"], np.float32)
    scales_param = np.asarray(inputs["scales_param"], np.float32)
    emb_W = np.asarray(inputs["emb_W"], np.float32)
    emb_b = np.asarray(inputs["emb_b"], np.float32)

    scales, im = _interp_host(masks, scales_param)
    coeff = (scales[0][None, :] * im * 2.0).astype(np.float32)  # [B, M]

    idn = np.eye(P)
    sel = np.zeros((P, 16, P), np.float16)
    for c_ in range(16):
        for p_ in range(P):
            sel[p_, c_, 8 * c_ + p_ // 16] = 1.0

    in_maps = []
    for core in range(NCORES):
        pair = core // 2
        rank = core % 2
        m = min(pair, 2)
        dup = pair == 3

        uniq, inv = np.unique(n_ids[m], return_inverse=True)
        nu = len(uniq)
        assert nu <= NUP - 1, nu
        WpreT = np.ascontiguousarray(pre_W[m].T).astype(np.float16)  # [IN_SIZE, HD]
        wlo_t = np.zeros((LO_ROWS, HD), np.float16)
        wlo_t[1:LO_ROWS] = WpreT[0:LO_ROWS - 1]
        whi_t = np.zeros((HI_ROWS, HD), np.float16)
        whi_t[1:IN_SIZE - (LO_ROWS - 1) + 1] = WpreT[LO_ROWS - 1:]
        lo_idx = np.zeros(NUP, np.int32)
        hi_idx = np.zeros(NUP, np.int32)
        lo_m = uniq < (LO_ROWS - 1)
        lo_idx[:nu][lo_m] = uniq[lo_m] + 1
        hi_idx[:nu][~lo_m] = uniq[~lo_m] - (LO_ROWS - 1) + 1

        d0 = rank * D0SH
        e0_sub = inv[e0_src[m]].reshape(N1, DEG)[d0:d0 + D0SH].reshape(-1)
        e1_all = e1_src[m]

        WgT = np.ascontiguousarray(gat_W[m].T)  # [HD, HD] (k, o)
        A = np.zeros((HD, HEADS), np.float32)
        for h in range(HEADS):
            A[h * DIM:(h + 1) * DIM, h] = gat_att[m, 0, h]
        WA = WgT @ A                      # [HD, HEADS]
        bwg_v = pre_b[m] @ WgT            # [HD]
        ba_v = bwg_v @ A                  # [HEADS]

        c0 = coeff[:, m].copy() if (rank == 0 and not dup) else np.zeros(B, np.float32)
        c1 = coeff[:, m].copy() if not dup else np.zeros(B, np.float32)
        if not dup:
            if rank == 0:
                c1[B // 2:] = 0.0
            else:
                c1[:B // 2] = 0.0

        embWT = np.ascontiguousarray(emb_W.T)  # [HD, P]

        in_maps.append(dict(
            wlo=wlo_t, whi=whi_t,
            galo=_wrap16(lo_idx, NUP), gahi=_wrap16(hi_idx, NUP),
            e0i=_wrap16(e0_sub, D0SH * DEG), e1i=_wrap16(e1_all, E1),
            wgt=np.stack([WgT[0:P], WgT[P:HD]]).astype(np.float16),
            wa=np.stack([WA[0:P], WA[P:HD]]).astype(np.float16),
            bwg=bwg_v[None, :].astype(np.float16),
            ba=ba_v[None, :].astype(np.float16),
            gbt=np.tile(gat_b[m][None, :], (P, 1)).astype(np.float32),
            ct0=c0.reshape(32, P).T.copy(),
            ct1=c1.reshape(32, P).T.copy(),
            embwt=np.stack([embWT[0:P], embWT[P:HD]]).astype(np.float32),
            embb=emb_b[:, None].astype(np.float32),
            idn16=idn.astype(np.float16), idn32=idn.astype(np.float32),
            sel=sel,
        ))
    return in_maps, scales


def kernel(**inputs):
    from concourse.bass_utils import run_bass_kernel_spmd
    if "nc" not in _CACHE:
        _CACHE["nc"] = _build_nc()
    nc = _CACHE["nc"]
    in_maps, scales = _host_prep(inputs)
    res = run_bass_kernel_spmd(nc, in_maps, list(range(NCORES)), trace=False)
    _CACHE["last"] = res
    dot = np.concatenate([res.results[c]["dot_sh"] for c in range(NCORES)], axis=0)
    emb = np.concatenate([res.results[c]["emb_sh"] for c in range(NCORES)], axis=0)
    return dot, emb, scales
